# revision 3
# baseline (speedup 1.0000x reference)
"""BiCGSTAB solver for nn_BiCG_Net on 8 TRN2 NeuronCores (pure data parallel).

Each core solves one (batch, channel) slice: a 384x384 variable-coefficient
5-point stencil system A x = b via BiCGSTAB with KMAX=30 iterations, restart
branch, and per-core convergence logic, entirely SBUF-resident.

Host side precomputes (numpy, f32): the transposed working-frame stencil
coefficients, the constant RHS c = mean(V)+1, initial x0/p0 and initial
scalars. Device runs the 30-iteration solve.

Grid layout on device: interior (384,384) row-major grid mapped to
[128 partitions, 1152 free]: row i = 128*g + p, col j -> [p, 384*g + j].
"""

import numpy as np

import concourse.bass as bass
import concourse.bacc as bacc
import concourse.mybir as mybir
import concourse.tile as tile
from concourse import bass_utils

F32 = np.float32
N = 384
GB = 3            # row-groups
P = 128           # partitions
W = GB * N        # 1152 free
KMAX = 30
EPS = 1e-9
THR2 = float(F32(EPS * N * N)) ** 2   # compare squared norms against thr^2

ET = mybir.EngineType

# ---------------- scalar slot indices in SC[128, NSLOT] ----------------
(RHO, R0ABS, RABS2, CC, VABS_E, RHS2, CONV, RES, NOTRES, FR1, FNR, NOTFNR,
 SIGS, ALPHA, ALPHAX, NALPHAX, C2, NOTC2, FC4, FC3, NOTFC4, TTS, OMEGA,
 OMEGAX, NOMEGAX, OMS, DENS, Q1, Q2, BETA, BETAX, NOTCONV, FPFIX,
 RECA, RECB, RECC) = range(36)
NSLOT = 36


# ======================= host-side precompute =======================

def _sym_pad2(a):
    return np.pad(a, ((1, 1), (1, 1)), mode='symmetric')


def host_prepare(V, mask1, mask2):
    """Per (b,c) slice: stencil coeffs (transposed working frame), x0, p0,
    initial scalars. All f32, mirroring the reference's op order."""
    Vt = np.ascontiguousarray(V.T)
    m1 = np.ascontiguousarray(mask1.T)
    m2 = np.ascontiguousarray(mask2.T)
    Vp = (_sym_pad2(Vt) + F32(1.0)).astype(F32)
    m1p = _sym_pad2(m1).astype(F32)
    m2p = _sym_pad2(m2).astype(F32)

    d1r = ((Vp[1:, :] - Vp[:-1, :]) / (F32(0.5) * (Vp[1:, :] + Vp[:-1, :]))).astype(F32)
    d2r = ((Vp[:, 1:] - Vp[:, :-1]) / (F32(0.5) * (Vp[:, 1:] + Vp[:, :-1]))).astype(F32)
    d1 = np.zeros((N + 2, N + 2), F32)
    d1[:N + 1, 1:N + 1] = d1r[:, 1:N + 1]
    d1 = (d1 * m1p).astype(F32)
    d2 = np.zeros((N + 2, N + 2), F32)
    d2[1:N + 1, :N + 1] = d2r[1:N + 1, :]
    d2 = (d2 * m2p).astype(F32)
    rx = F32(5.0)
    rxx = F32(10.0)
    dd1 = (np.pad(d1, ((1, 0), (0, 0)))[:-1, :] - d1).astype(F32)
    dd2 = (np.pad(d2, ((0, 0), (1, 0)))[:, :-1] - d2).astype(F32)
    boo = (F32(1.0) + F32(2.0) * (rxx + rxx) - rx * dd1 - rx * dd2)[1:N + 1, 1:N + 1].astype(F32)
    bpo = (-rxx + rx * d1[1:N + 1, 1:N + 1]).astype(F32)
    bop = (-rxx + rx * d2[1:N + 1, 1:N + 1]).astype(F32)
    bmo = (-rxx - rx * d1[:N, 1:N + 1]).astype(F32)
    bom = (-rxx - rx * d2[1:N + 1, :N]).astype(F32)

    c = F32(np.mean(V, dtype=F32) + F32(1.0))
    # p0 = b - A(x0) with x0 = c everywhere (symmetric pad keeps neighbors = c)
    ax0 = ((((boo * c + bmo * c) + bom * c) + bop * c) + bpo * c).astype(F32)
    p0 = (c - ax0).astype(F32)
    x0 = np.full((N, N), c, F32)
    rho0 = F32(np.sum((p0 * p0).astype(F32), dtype=F32))
    r0abs = F32(np.sqrt(rho0))
    return dict(boo=boo, bmo=bmo, bom=bom, bop=bop, bpo=bpo,
                x0=x0, p0=p0, c=c, rho0=rho0, r0abs=r0abs)


def to_dev(a):
    """(384,384) -> [128, 1152] layout."""
    return np.ascontiguousarray(a.reshape(GB, P, N).transpose(1, 0, 2).reshape(P, W))


def from_dev(a):
    """[128, 1152] -> (384,384)."""
    return np.ascontiguousarray(a.reshape(P, GB, N).transpose(1, 0, 2).reshape(N, N))


# ======================= device program =======================

def _apply_A(nc, cf, sh, z, out, T1, T2, T3, T4, zups, zdps):
    """out = A(z), 5-point stencil with symmetric-edge replication.

    z/out/temps are [128, 1152] SBUF tiles; cf = coefficient tiles; sh =
    dict of PE shift matrices; zups/zdps = [128, 3*512] PSUM tiles holding
    the partition-shifted copies of z (block g at free cols [512g, 512g+384)).

    Compute engines can't read partition-shifted SBUF APs (start partition
    must be 0/32/64/96), so the i+-1 shifts go through the TensorEngine:
    zu = Su @ z per 384-col block, with cross-block boundary rows added via
    PSUM accumulation of a rank-1 matmul.
    """
    boo, bmo, bom, bop = cf['boo'], cf['bmo'], cf['bom'], cf['bop']
    bpo = cf['bpo']
    z3 = z[:].rearrange("p (g w) -> p g w", g=GB)
    bom3 = bom[:].rearrange("p (g w) -> p g w", g=GB)
    bop3 = bop[:].rearrange("p (g w) -> p g w", g=GB)
    T33 = T3[:].rearrange("p (g w) -> p g w", g=GB)
    T43 = T4[:].rearrange("p (g w) -> p g w", g=GB)
    zu3 = zups[:].rearrange("p (g w) -> p g w", g=GB)  # w = 512
    zd3 = zdps[:].rearrange("p (g w) -> p g w", g=GB)

    # ---- PE: zu[i] = z[i-1] (row 0 replicated), zd[i] = z[i+1] ----
    for g in range(GB):
        lhs = sh['Su0'] if g == 0 else sh['SuM']
        nc.tensor.matmul(zu3[:, g, 0:N], lhs[:, :], z3[:, g, :],
                         start=True, stop=(g == 0))
        if g > 0:
            nc.tensor.matmul(zu3[:, g, 0:N], sh['SuX'][:, :], z3[:, g - 1, :],
                             start=False, stop=True)
    for g in range(GB):
        lhs = sh['Sd2'] if g == GB - 1 else sh['SdM']
        nc.tensor.matmul(zd3[:, g, 0:N], lhs[:, :], z3[:, g, :],
                         start=True, stop=(g == GB - 1))
        if g < GB - 1:
            nc.tensor.matmul(zd3[:, g, 0:N], sh['SdX'][:, :], z3[:, g + 1, :],
                             start=False, stop=True)

    # center
    nc.gpsimd.tensor_mul(out[:, :], boo[:, :], z[:, :])
    # up/down products (PSUM operand -> DVE only)
    T13 = T1[:].rearrange("p (g w) -> p g w", g=GB)
    T23 = T2[:].rearrange("p (g w) -> p g w", g=GB)
    bmo3 = bmo[:].rearrange("p (g w) -> p g w", g=GB)
    bpo3 = bpo[:].rearrange("p (g w) -> p g w", g=GB)
    nc.vector.tensor_mul(T13[:, :, :], bmo3[:, :, :], zu3[:, :, 0:N])
    nc.vector.tensor_mul(T23[:, :, :], bpo3[:, :, :], zd3[:, :, 0:N])
    # left (j-1): free-dim shift, 3D AP
    nc.gpsimd.tensor_mul(T33[:, :, 1:N], bom3[:, :, 1:N], z3[:, :, 0:N - 1])
    nc.vector.tensor_mul(T33[:, :, 0:1], bom3[:, :, 0:1], z3[:, :, 0:1])
    # right (j+1)
    nc.gpsimd.tensor_mul(T43[:, :, 0:N - 1], bop3[:, :, 0:N - 1], z3[:, :, 1:N])
    nc.vector.tensor_mul(T43[:, :, N - 1:N], bop3[:, :, N - 1:N], z3[:, :, N - 1:N])
    # accumulate: out += (T1+T2) + (T3+T4)
    nc.vector.tensor_add(T1[:, :], T1[:, :], T2[:, :])
    nc.gpsimd.tensor_add(T3[:, :], T3[:, :], T4[:, :])
    nc.vector.tensor_add(out[:, :], out[:, :], T1[:, :])
    nc.gpsimd.tensor_add(out[:, :], out[:, :], T3[:, :])


def build_nc(kmax=KMAX, use_ifs=True):
    nc = bacc.Bacc("TRN2", debug=False, num_devices=8)
    dt = mybir.dt.float32
    din = {}
    for nm in ("boo", "bmo", "bom", "bop", "bpo", "x0", "p0"):
        din[nm] = nc.dram_tensor(nm, [P, W], dt, kind="ExternalInput").ap()
    scal_in = nc.dram_tensor("scal", [P, 4], dt, kind="ExternalInput").ap()
    sh_in = nc.dram_tensor("shifts", [6, P, P], dt, kind="ExternalInput").ap()
    xout = nc.dram_tensor("xout", [P, W], dt, kind="ExternalOutput").ap()
    SH_NAMES = ("SuM", "Su0", "SuX", "SdM", "Sd2", "SdX")

    with tile.TileContext(nc) as tc:
        import contextlib
        with contextlib.ExitStack() as ctx:
            big = ctx.enter_context(tc.tile_pool(name="big", bufs=1))
            small = ctx.enter_context(tc.tile_pool(name="small", bufs=1))
            psum = ctx.enter_context(tc.tile_pool(name="psum", bufs=1, space="PSUM"))

            cf = {nm: big.tile([P, W], dt, tag=nm, name=nm) for nm in
                  ("boo", "bmo", "bom", "bop", "bpo")}
            x = big.tile([P, W], dt, tag="x")
            r = big.tile([P, W], dt, tag="r")
            r0 = big.tile([P, W], dt, tag="r0")
            pA = big.tile([P, W], dt, tag="pA")
            pB = big.tile([P, W], dt, tag="pB")
            v = big.tile([P, W], dt, tag="v")
            s = big.tile([P, W], dt, tag="s")
            t = big.tile([P, W], dt, tag="t")
            u = big.tile([P, W], dt, tag="u")
            T1 = big.tile([P, W], dt, tag="T1")
            T2 = big.tile([P, W], dt, tag="T2")
            T3 = big.tile([P, W], dt, tag="T3")
            T4 = big.tile([P, W], dt, tag="T4")
            TS1 = big.tile([P, W], dt, tag="TS1")
            TS2 = big.tile([P, W], dt, tag="TS2")

            SC = small.tile([P, NSLOT], dt, tag="SC")
            PT = small.tile([P, 8], dt, tag="PT")
            ones = small.tile([P, P], dt, tag="ones")
            sh = {nm: small.tile([P, P], dt, tag=nm, name=nm) for nm in SH_NAMES}

            ps_dots = psum.tile([P, 8], dt, tag="ps_dots")
            zups = psum.tile([P, 3 * 512], dt, tag="zups")
            zdps = psum.tile([P, 3 * 512], dt, tag="zdps")

            def S(k):
                return SC[:, k:k + 1]

            AF = mybir.ActivationFunctionType
            OP = mybir.AluOpType

            def act(dst, src, func=AF.Identity, bias=0.0, scale=1.0, accum=None):
                nc.scalar.activation(dst, src, func, bias=bias, scale=scale,
                                     accum_out=accum)

            # ---- loads ----
            nc.sync.dma_start(SC[:, 0:4], scal_in)
            for nm in ("boo", "bmo", "bom", "bop", "bpo"):
                nc.sync.dma_start(cf[nm][:, :], din[nm])
            nc.sync.dma_start(x[:, :], din["x0"])
            nc.sync.dma_start(pA[:, :], din["p0"])
            for i, nm in enumerate(SH_NAMES):
                nc.sync.dma_start(sh[nm][:, :], sh_in[i])
            nc.vector.memset(ones[:, :], 1.0)
            nc.vector.tensor_copy(r[:, :], pA[:, :])
            nc.scalar.copy(r0[:, :], pA[:, :])

            # branch registers (allocated once, reused every iteration)
            regs_r1 = nc.alloc_registers(
                "fr1", bass.OrderedSet([ET.DVE, ET.Pool, ET.Activation, ET.PE]))
            regs_c3 = nc.alloc_registers(
                "fc3", bass.OrderedSet([ET.DVE, ET.PE]))
            regs_fix = nc.alloc_registers("ffix", bass.OrderedSet([ET.DVE]))

            pcur, pnxt = pA, pB
            for it in range(kmax):
                # ---------- v = A(p) ----------
                _apply_A(nc, cf, sh, pcur, v, T1, T2, T3, T4, zups, zdps)
                # ---------- sigma = <v, r0>, vv = <v, v> ----------
                nc.vector.scalar_tensor_tensor(
                    out=TS1[:, :], in0=v[:, :], scalar=1.0, in1=r0[:, :],
                    op0=OP.mult, op1=OP.mult, accum_out=PT[:, 0:1])
                act(TS2[:, :], v[:, :], AF.Square, accum=PT[:, 1:2])
                nc.tensor.matmul(ps_dots[:, 0:2], ones[:, :], PT[:, 0:2],
                                 start=True, stop=True)
                # ---------- flags ----------
                act(S(VABS_E), ps_dots[:, 1:2], AF.Sqrt, scale=float(EPS) * float(EPS))
                nc.vector.tensor_mul(S(RHS2), S(VABS_E), S(R0ABS))
                nc.vector.tensor_scalar(out=S(CONV), in0=S(RABS2),
                                        scalar1=THR2, scalar2=None, op0=OP.is_gt)
                nc.vector.tensor_tensor(out=S(RES), in0=ps_dots[:, 0:1],
                                        in1=S(RHS2), op=OP.is_le)
                nc.vector.tensor_mul(S(FR1), S(CONV), S(RES))
                act(S(NOTRES), S(RES), scale=-1.0, bias=1.0)
                act(S(FNR), S(CONV), scale=S(NOTRES))

                # ---------- restart branch (rare) ----------
                if use_ifs:
                  for reg in regs_r1:
                    nc.reg_load(reg, SC[0:1, FR1:FR1 + 1].bitcast(mybir.dt.uint32))
                if use_ifs:
                 with tc.If(nc.snap(regs_r1, donate=True) > 0):
                    _apply_A(nc, cf, sh, x, u, T1, T2, T3, T4, zups, zdps)
                    # r = c - A(x);  r0 = r;  rho = rabs2 = <r,r>; r0abs = sqrt
                    nc.vector.tensor_scalar(out=r[:, :], in0=u[:, :],
                                            scalar1=-1.0, scalar2=S(CC),
                                            op0=OP.mult, op1=OP.add)
                    act(r0[:, :], r[:, :], AF.Copy)
                    act(TS2[:, :], r[:, :], AF.Square, accum=PT[:, 7:8])
                    nc.tensor.matmul(ps_dots[:, 7:8], ones[:, :], PT[:, 7:8],
                                     start=True, stop=True)
                    nc.vector.tensor_copy(S(RHO), ps_dots[:, 7:8])
                    nc.vector.tensor_copy(S(RABS2), ps_dots[:, 7:8])
                    act(S(R0ABS), ps_dots[:, 7:8], AF.Sqrt)

                # ---------- alpha ----------
                act(S(NOTFNR), S(FNR), scale=-1.0, bias=1.0)
                act(S(SIGS), ps_dots[:, 0:1], scale=S(FNR), bias=S(NOTFNR))
                nc.vector.reciprocal(S(RECA), S(SIGS))
                nc.vector.tensor_mul(S(ALPHA), S(RHO), S(RECA))
                act(S(ALPHAX), S(ALPHA), scale=S(FNR))
                act(S(NALPHAX), S(ALPHAX), scale=-1.0)
                # ---------- s = r - alpha*v ----------
                nc.vector.scalar_tensor_tensor(
                    out=s[:, :], in0=v[:, :], scalar=S(NALPHAX), in1=r[:, :],
                    op0=OP.mult, op1=OP.add)
                # ---------- ss, C2, fc3/fc4 ----------
                act(TS2[:, :], s[:, :], AF.Square, accum=PT[:, 2:3])
                nc.tensor.matmul(ps_dots[:, 2:3], ones[:, :], PT[:, 2:3],
                                 start=True, stop=True)
                nc.vector.tensor_scalar(out=S(C2), in0=ps_dots[:, 2:3],
                                        scalar1=THR2, scalar2=None, op0=OP.is_le)
                act(S(NOTC2), S(C2), scale=-1.0, bias=1.0)
                act(S(FC4), S(FNR), scale=S(NOTC2))
                act(S(FC3), S(FNR), scale=S(C2))
                # ---------- t = A(s) ----------
                _apply_A(nc, cf, sh, s, t, T1, T2, T3, T4, zups, zdps)
                # ---------- ts, tt ----------
                nc.vector.scalar_tensor_tensor(
                    out=TS1[:, :], in0=t[:, :], scalar=1.0, in1=s[:, :],
                    op0=OP.mult, op1=OP.mult, accum_out=PT[:, 3:4])
                act(TS2[:, :], t[:, :], AF.Square, accum=PT[:, 4:5])
                nc.tensor.matmul(ps_dots[:, 3:5], ones[:, :], PT[:, 3:5],
                                 start=True, stop=True)
                # ---------- omega ----------
                act(S(NOTFC4), S(FC4), scale=-1.0, bias=1.0)
                act(S(TTS), ps_dots[:, 4:5], scale=S(FC4), bias=S(NOTFC4))
                nc.vector.reciprocal(S(RECB), S(TTS))
                nc.vector.tensor_mul(S(OMEGA), ps_dots[:, 3:4], S(RECB))
                act(S(OMEGAX), S(OMEGA), scale=S(FC4))
                act(S(NOMEGAX), S(OMEGAX), scale=-1.0)
                # ---------- x += alpha*p + omega*s (off critical path) ----------
                nc.vector.scalar_tensor_tensor(
                    out=x[:, :], in0=pcur[:, :], scalar=S(ALPHAX), in1=x[:, :],
                    op0=OP.mult, op1=OP.add)
                nc.vector.scalar_tensor_tensor(
                    out=x[:, :], in0=s[:, :], scalar=S(OMEGAX), in1=x[:, :],
                    op0=OP.mult, op1=OP.add)
                # ---------- r = s - omega*t ----------
                nc.vector.scalar_tensor_tensor(
                    out=r[:, :], in0=t[:, :], scalar=S(NOMEGAX), in1=s[:, :],
                    op0=OP.mult, op1=OP.add)
                # ---------- rho' = <r, r0>, rr = <r, r> ----------
                nc.vector.scalar_tensor_tensor(
                    out=TS1[:, :], in0=r[:, :], scalar=1.0, in1=r0[:, :],
                    op0=OP.mult, op1=OP.mult, accum_out=PT[:, 5:6])
                act(TS2[:, :], r[:, :], AF.Square, accum=PT[:, 6:7])
                nc.tensor.matmul(ps_dots[:, 5:7], ones[:, :], PT[:, 5:7],
                                 start=True, stop=True)
                # ---------- beta ----------
                act(S(OMS), S(OMEGAX), bias=S(NOTFC4))
                act(S(DENS), S(RHO), scale=S(FC4), bias=S(NOTFC4))
                nc.vector.reciprocal(S(RECC), S(OMS))
                nc.vector.tensor_mul(S(Q1), S(ALPHA), S(RECC))
                nc.vector.reciprocal(S(RECA), S(DENS))
                nc.vector.tensor_mul(S(Q2), ps_dots[:, 5:6], S(RECA))
                act(S(BETA), S(Q1), scale=S(Q2))
                act(S(BETAX), S(BETA), scale=S(FC4))
                # ---------- p' = r + betax*(p - omegax*v) ----------
                nc.vector.scalar_tensor_tensor(
                    out=u[:, :], in0=v[:, :], scalar=S(NOMEGAX), in1=pcur[:, :],
                    op0=OP.mult, op1=OP.add)
                nc.vector.scalar_tensor_tensor(
                    out=pnxt[:, :], in0=u[:, :], scalar=S(BETAX), in1=r[:, :],
                    op0=OP.mult, op1=OP.add)
                # ---------- scalar state updates ----------
                nc.vector.copy_predicated(S(RHO), S(FC4).bitcast(mybir.dt.uint32), ps_dots[:, 5:6])
                nc.vector.copy_predicated(S(RABS2), S(FC4).bitcast(mybir.dt.uint32), ps_dots[:, 6:7])
                # ---------- p fixup when frozen or C3 (rare/never) ----------
                act(S(NOTCONV), S(CONV), scale=-1.0, bias=1.0)
                act(S(FPFIX), S(FC3), bias=S(NOTCONV))
                if use_ifs:
                  for reg in regs_fix:
                    nc.reg_load(reg, SC[0:1, FPFIX:FPFIX + 1].bitcast(mybir.dt.uint32))
                  with tc.If(nc.snap(regs_fix, donate=True) > 0):
                    nc.vector.tensor_copy(pnxt[:, :], pcur[:, :])
                # ---------- C3 scalar fixups (never in practice) ----------
                if use_ifs:
                  for reg in regs_c3:
                    nc.reg_load(reg, SC[0:1, FC3:FC3 + 1].bitcast(mybir.dt.uint32))
                  with tc.If(nc.snap(regs_c3, donate=True) > 0):
                    nc.vector.scalar_tensor_tensor(
                        out=TS1[:, :], in0=s[:, :], scalar=1.0, in1=r0[:, :],
                        op0=OP.mult, op1=OP.mult, accum_out=PT[:, 7:8])
                    nc.tensor.matmul(ps_dots[:, 7:8], ones[:, :], PT[:, 7:8],
                                     start=True, stop=True)
                    nc.vector.tensor_copy(S(RHO), ps_dots[:, 7:8])
                    nc.vector.tensor_copy(S(RABS2), ps_dots[:, 2:3])

                pcur, pnxt = pnxt, pcur

            nc.sync.dma_start(xout, x[:, :])
    nc.compile()
    return nc


# ======================= public entry point =======================

def make_shift_mats():
    """PE shift matrices (lhsT layout [k, m]: out[m] = sum_k lhsT[k,m] z[k])."""
    SuM = np.zeros((P, P), F32)   # out[m] = z[m-1]
    for m in range(1, P):
        SuM[m - 1, m] = 1.0
    Su0 = SuM.copy()              # + replicate row 0 (top edge of grid)
    Su0[0, 0] = 1.0
    SuX = np.zeros((P, P), F32)   # out[0] = z[127] (previous block)
    SuX[P - 1, 0] = 1.0
    SdM = np.zeros((P, P), F32)   # out[m] = z[m+1]
    for m in range(P - 1):
        SdM[m + 1, m] = 1.0
    Sd2 = SdM.copy()              # + replicate row 127 (bottom edge)
    Sd2[P - 1, P - 1] = 1.0
    SdX = np.zeros((P, P), F32)   # out[127] = z[0] (next block)
    SdX[0, P - 1] = 1.0
    return np.stack([SuM, Su0, SuX, SdM, Sd2, SdX])


_CACHE = {}


def make_in_map(V2d, m1_2d, m2_2d):
    """Per-core input map from one (384,384) slice."""
    h = host_prepare(np.asarray(V2d, F32), np.asarray(m1_2d, F32),
                     np.asarray(m2_2d, F32))
    scal = np.zeros((P, 4), F32)
    scal[:, 0] = h["rho0"]
    scal[:, 1] = h["r0abs"]
    scal[:, 2] = h["rho0"]      # r_abs^2 = rho0 initially
    scal[:, 3] = h["c"]
    return {
        "boo": to_dev(h["boo"]), "bmo": to_dev(h["bmo"]),
        "bom": to_dev(h["bom"]), "bop": to_dev(h["bop"]),
        "bpo": to_dev(h["bpo"]), "x0": to_dev(h["x0"]),
        "p0": to_dev(h["p0"]), "scal": scal, "shifts": make_shift_mats(),
    }


def kernel(V, mask1, mask2):
    B, C = V.shape[0], V.shape[1]
    assert (B, C) == (8, 1) and V.shape[2:] == (N, N)
    if "nc" not in _CACHE:
        _CACHE["nc"] = build_nc()
    nc = _CACHE["nc"]

    in_maps = [make_in_map(V[b, 0], mask1[b, 0], mask2[b, 0]) for b in range(B)]

    res = bass_utils.run_bass_kernel_spmd(nc, in_maps, core_ids=list(range(8)))
    global LAST_RES
    LAST_RES = res
    out = np.empty((B, C, N, N), F32)
    for b in range(B):
        out[b, 0] = from_dev(res.results[b]["xout"])
    return out


if __name__ == "__main__":
    rng = np.random.default_rng(0)
    V = rng.random((8, 1, N, N), F32)
    m1 = rng.random((8, 1, N, N), F32)
    m2 = rng.random((8, 1, N, N), F32)
    out = kernel(V, m1, m2)
    print("kernel ran:", out.shape, out.dtype, float(np.abs(out).mean()))

